# revision 2
# baseline (speedup 1.0000x reference)
"""GAU (Gated Attention Unit) forward on 8 Trainium2 NeuronCores.

Data-parallel over batch: B=32 -> 4 batch elements per core, every core runs
the identical program on its batch shard with full (replicated) weights.

Per-core schedule (two weight-residency phases so SBUF fits):
  Phase 1 (uv_w.T qkv columns resident):
    LayerNorm -> PE-transpose xn -> qkv projection -> rope (PE half-swap) ->
    scoresT (+Toeplitz bias via identity matmul) -> relu^2 -> attn @ v.
    Spills xnT and aT (attention output, transposed) to DRAM scratch.
  Phase 2 (uv_w.T u columns + o_w.T resident):
    u projection -> silu -> gate (u * a) -> output projection + residual.

All matmuls run in float32r (full-rate PE) with fp32 PSUM accumulation.
LayerNorm weight/bias and all linear biases are folded on the host into the
weight matrices / per-partition activation biases, so the device kernel only
does the irreducible work.
"""

import numpy as np
from contextlib import ExitStack

import concourse.bass as bass
import concourse.tile as tile
from concourse import bacc, mybir
from concourse.bass_utils import run_bass_kernel_spmd
from concourse.masks import make_identity

F32 = mybir.dt.float32
F32R = mybir.dt.float32r
AF = mybir.ActivationFunctionType
OP = mybir.AluOpType

B, T, H, E, S, L = 32, 512, 1024, 2048, 128, 512
NCORES = 8
BPC = B // NCORES          # batch elements per core
EPS = 1e-5
HC = H // 128              # 8 H-chunks
EC = E // 128              # 16 E-chunks
TC = T // 128              # 4 token chunks
NQKV = E + S               # 2176 columns of the fused qkv part (v then base)


def _emit(nc, with_vbias):
    """Emit the per-core tile program."""
    x_d = nc.dram_tensor("x_in", [BPC, T, H], F32, kind="ExternalInput")
    xres_d = nc.dram_tensor("xres_in", [BPC, T, H], F32, kind="ExternalInput")
    wqkv_d = nc.dram_tensor("wqkv_in", [H, NQKV], F32R, kind="ExternalInput")
    wu_d = nc.dram_tensor("wu_in", [H, E], F32R, kind="ExternalInput")
    wo_d = nc.dram_tensor("wo_in", [E, H], F32R, kind="ExternalInput")
    biasT_d = nc.dram_tensor("biasT_in", [T, T], F32R, kind="ExternalInput")
    ropeC_d = nc.dram_tensor("ropeC_in", [S, T], F32, kind="ExternalInput")
    ropeS_d = nc.dram_tensor("ropeS_in", [S, T], F32, kind="ExternalInput")
    gb_d = nc.dram_tensor("gb_in", [S, 4], F32, kind="ExternalInput")
    ubu_d = nc.dram_tensor("ubu_in", [128, EC], F32, kind="ExternalInput")
    ubb_d = nc.dram_tensor("ubb_in", [S, 1], F32, kind="ExternalInput")
    vb_d = nc.dram_tensor("vb_in", [1, E], F32R, kind="ExternalInput")
    y_d = nc.dram_tensor("y_out", [BPC, T, H], F32, kind="ExternalOutput")

    with tile.TileContext(nc) as tc, ExitStack() as ctx:
        # ---------------- constant pools (whole-kernel lifetime) ----------
        consts = ctx.enter_context(tc.tile_pool(name="consts", bufs=1))
        ident_f = consts.tile([128, 128], F32, tag="ident_f")
        make_identity(nc, ident_f)
        ident = consts.tile([128, 128], F32R, tag="ident")
        nc.vector.tensor_copy(out=ident[:], in_=ident_f[:])
        perm_f = consts.tile([128, 128], F32, tag="perm_f")
        nc.gpsimd.memset(perm_f, 0.0)
        for base in (-64, 64):
            nc.gpsimd.affine_select(
                out=perm_f, in_=perm_f, compare_op=OP.not_equal,
                fill=1.0, base=base, pattern=[[-1, 128]], channel_multiplier=1,
            )
        perm = consts.tile([128, 128], F32R, tag="perm")
        nc.vector.tensor_copy(out=perm[:], in_=perm_f[:])
        gb = consts.tile([S, 4], F32, tag="gb")
        nc.sync.dma_start(out=gb, in_=gb_d[:])
        ubu = consts.tile([128, EC], F32, tag="ubu")
        nc.sync.dma_start(out=ubu, in_=ubu_d[:])
        ubb = consts.tile([S, 1], F32, tag="ubb")
        nc.sync.dma_start(out=ubb, in_=ubb_d[:])
        eps_t = consts.tile([128, 1], F32, tag="eps")
        nc.vector.memset(eps_t, EPS)
        if with_vbias:
            ones_row = consts.tile([1, 128], F32R, tag="ones_row")
            of = consts.tile([1, 128], F32, tag="ones_row_f")
            nc.vector.memset(of, 1.0)
            nc.vector.tensor_copy(out=ones_row[:], in_=of[:])
            vb_row = consts.tile([1, E], F32R, tag="vb_row")
            nc.sync.dma_start(out=vb_row, in_=vb_d[:])

        # DRAM scratch for phase-1 -> phase-2 handoff
        dram = ctx.enter_context(tc.tile_pool(name="dram", bufs=1, space="DRAM"))
        xnT_spill = dram.tile([BPC, HC, 128, T], F32R)
        aT_spill = dram.tile([BPC, EC, 128, T], F32)

        # ================= PHASE 1 =================
        with ExitStack() as p1:
            p1c = p1.enter_context(tc.tile_pool(name="p1consts", bufs=1))
            biasT = p1c.tile([128, TC, T], F32R, tag="biasT")
            nc.sync.dma_start(
                out=biasT, in_=biasT_d[:].rearrange("(c p) i -> p c i", p=128))
            ropeC = p1c.tile([S, T], F32, tag="ropeC")
            nc.sync.dma_start(out=ropeC, in_=ropeC_d[:])
            ropeS = p1c.tile([S, T], F32, tag="ropeS")
            nc.sync.dma_start(out=ropeS, in_=ropeS_d[:])

            p1w = p1.enter_context(tc.tile_pool(name="p1w", bufs=1))
            wqkv = p1w.tile([128, HC, NQKV], F32R, tag="wqkv")
            nc.sync.dma_start(
                out=wqkv, in_=wqkv_d[:].rearrange("(c p) f -> p c f", p=128))

            xp = p1.enter_context(tc.tile_pool(name="xp", bufs=2))
            xnp = p1.enter_context(tc.tile_pool(name="xnp", bufs=4))
            xnTp = p1.enter_context(tc.tile_pool(name="xnTp", bufs=1))
            vp = p1.enter_context(tc.tile_pool(name="vp", bufs=4))
            rw = p1.enter_context(tc.tile_pool(name="rw", bufs=2))
            ktp = p1.enter_context(tc.tile_pool(name="ktp", bufs=4))
            atp = p1.enter_context(tc.tile_pool(name="atp", bufs=2))
            ps1 = p1.enter_context(tc.tile_pool(name="ps1", bufs=3, space="PSUM"))
            pst = p1.enter_context(tc.tile_pool(name="pst", bufs=2, space="PSUM"))

            for e in range(BPC):
                # ---- LayerNorm (tokens on partitions) ----
                xn_tiles = []
                for tci in range(TC):
                    xt = xp.tile([128, H], F32, tag="x")
                    nc.sync.dma_start(
                        out=xt, in_=x_d[e, tci * 128:(tci + 1) * 128, :])
                    st = rw.tile([128, 2, 6], F32, tag="bnst", bufs=4)
                    xv = xt[:].rearrange("p (g d) -> p g d", g=2)
                    nc.vector.bn_stats(out=st[:, 0, :], in_=xv[:, 0, :])
                    nc.vector.bn_stats(out=st[:, 1, :], in_=xv[:, 1, :])
                    mv = rw.tile([128, 2], F32, tag="bnmv", bufs=4)
                    nc.vector.bn_aggr(out=mv[:], in_=st[:])
                    std = rw.tile([128, 1], F32, tag="std")
                    nc.scalar.activation(
                        out=std[:], in_=mv[:, 1:2], func=AF.Sqrt,
                        bias=eps_t[:], scale=1.0)
                    rstd = rw.tile([128, 1], F32, tag="rstd")
                    nc.vector.reciprocal(out=rstd[:], in_=std[:])
                    xn = xnp.tile([128, H], F32R, tag="xn")
                    nc.vector.tensor_scalar(
                        out=xn[:], in0=xt[:], scalar1=mv[:, 0:1],
                        scalar2=rstd[:], op0=OP.subtract, op1=OP.mult)
                    xn_tiles.append(xn)

                # ---- transpose xn -> xnT [128h x (HC, T)] ----
                xnT = xnTp.tile([128, HC, T], F32R, tag="xnT")
                for hc in range(HC):
                    tps = pst.tile([128, TC, 128], F32R, tag="tps")
                    for tci in range(TC):
                        nc.tensor.transpose(
                            tps[:, tci, :],
                            xn_tiles[tci][:, hc * 128:(hc + 1) * 128],
                            ident[:])
                    nc.vector.tensor_copy(out=xnT[:, hc, :], in_=tps[:])
                nc.sync.dma_start(
                    out=xnT_spill[e].rearrange("c p t -> p c t"), in_=xnT[:])

                # ---- base projection -> [s, t], silu, gamma/beta, rope ----
                bps = ps1.tile([128, T], F32, tag="ps")
                for k in range(HC):
                    nc.tensor.matmul(
                        bps[:], wqkv[:, k, E:E + S], xnT[:, k, :],
                        start=(k == 0), stop=(k == HC - 1))
                ubT = rw.tile([S, T], F32, tag="ubT")
                nc.scalar.activation(
                    out=ubT[:], in_=bps[:], func=AF.Silu, bias=ubb[:], scale=1.0)

                qT = None
                kT = None
                for qi in (0, 1):  # 0 -> q (gamma/beta pre-scaled by 1/sqrt(S))
                    pre = rw.tile([S, T], F32R, tag="pre")
                    nc.vector.tensor_scalar(
                        out=pre[:], in0=ubT[:],
                        scalar1=gb[:, 2 * qi:2 * qi + 1],
                        scalar2=gb[:, 2 * qi + 1:2 * qi + 2],
                        op0=OP.mult, op1=OP.add)
                    sps = ps1.tile([128, T], F32, tag="ps")
                    nc.tensor.matmul(sps[:], perm[:], pre[:], start=True, stop=True)
                    t1 = rw.tile([S, T], F32, tag="t1")
                    nc.vector.tensor_tensor(
                        out=t1[:], in0=pre[:], in1=ropeC[:], op=OP.mult)
                    t2 = rw.tile([S, T], F32, tag="t2")
                    nc.vector.tensor_tensor(
                        out=t2[:], in0=sps[:], in1=ropeS[:], op=OP.mult)
                    qkt = rw.tile([S, T], F32R, tag="qkt", bufs=3)
                    nc.vector.tensor_tensor(
                        out=qkt[:], in0=t1[:], in1=t2[:], op=OP.add)
                    if qi == 0:
                        qT = qkt
                    else:
                        kT = qkt

                # ---- v projection -> [t, e] ----
                v_tiles = []
                for tci in range(TC):
                    vt = vp.tile([128, E], F32R, tag="v")
                    for fs in range(E // 512):
                        vps = ps1.tile([128, 512], F32, tag="ps")
                        for k in range(HC):
                            nc.tensor.matmul(
                                vps[:],
                                xnT[:, k, tci * 128:(tci + 1) * 128],
                                wqkv[:, k, fs * 512:(fs + 1) * 512],
                                start=(k == 0),
                                stop=(k == HC - 1) and not with_vbias)
                        if with_vbias:
                            nc.tensor.matmul(
                                vps[:], ones_row[:],
                                vb_row[:, fs * 512:(fs + 1) * 512],
                                start=False, stop=True)
                        nc.scalar.activation(
                            out=vt[:, fs * 512:(fs + 1) * 512], in_=vps[:],
                            func=AF.Silu)
                    v_tiles.append(vt)

                # ---- scoresT + Toeplitz bias, relu^2 ----
                kern_tiles = []
                for jc in range(TC):
                    scps = ps1.tile([128, T], F32, tag="ps")
                    nc.tensor.matmul(
                        scps[:], kT[:, jc * 128:(jc + 1) * 128], qT[:],
                        start=True, stop=False)
                    nc.tensor.matmul(
                        scps[:], ident[:], biasT[:, jc, :],
                        start=False, stop=True)
                    krl = rw.tile([128, T], F32, tag="krl")
                    nc.scalar.activation(out=krl[:], in_=scps[:], func=AF.Relu)
                    kt = ktp.tile([128, T], F32R, tag="kern")
                    nc.vector.tensor_tensor(
                        out=kt[:], in0=krl[:], in1=krl[:], op=OP.mult)
                    kern_tiles.append(kt)

                # ---- attention output (transposed): aT[e_chunk, t] ----
                for ec in range(EC):
                    aps = ps1.tile([128, T], F32, tag="ps")
                    for jc in range(TC):
                        nc.tensor.matmul(
                            aps[:],
                            v_tiles[jc][:, ec * 128:(ec + 1) * 128],
                            kern_tiles[jc][:],
                            start=(jc == 0), stop=(jc == TC - 1))
                    at = atp.tile([128, T], F32, tag="aT")
                    nc.any.tensor_copy(out=at[:], in_=aps[:])
                    nc.sync.dma_start(out=aT_spill[e, ec], in_=at[:])

        # ================= PHASE 2 =================
        with ExitStack() as p2:
            p2w = p2.enter_context(tc.tile_pool(name="p2w", bufs=1))
            wu = p2w.tile([128, HC, E], F32R, tag="wu")
            nc.sync.dma_start(
                out=wu, in_=wu_d[:].rearrange("(c p) f -> p c f", p=128))
            wo = p2w.tile([128, EC, H], F32R, tag="wo")
            nc.sync.dma_start(
                out=wo, in_=wo_d[:].rearrange("(c p) h -> p c h", p=128))

            xnT2p = p2.enter_context(tc.tile_pool(name="xnT2p", bufs=1))
            utp = p2.enter_context(tc.tile_pool(name="utp", bufs=2))
            at2p = p2.enter_context(tc.tile_pool(name="at2p", bufs=2))
            gtp = p2.enter_context(tc.tile_pool(name="gtp", bufs=EC))
            yp = p2.enter_context(tc.tile_pool(name="yp", bufs=2))
            xrp = p2.enter_context(tc.tile_pool(name="xrp", bufs=2))
            ps2 = p2.enter_context(tc.tile_pool(name="ps2", bufs=3, space="PSUM"))

            for e in range(BPC):
                xnT2 = xnT2p.tile([128, HC, T], F32R, tag="xnT2")
                nc.sync.dma_start(
                    out=xnT2, in_=xnT_spill[e].rearrange("c p t -> p c t"))

                # ---- u projection -> [e_chunk, t], silu, gate with aT ----
                g_tiles = []
                for ec in range(EC):
                    ups = ps2.tile([128, T], F32, tag="ps")
                    for k in range(HC):
                        nc.tensor.matmul(
                            ups[:], wu[:, k, ec * 128:(ec + 1) * 128],
                            xnT2[:, k, :],
                            start=(k == 0), stop=(k == HC - 1))
                    ut = utp.tile([128, T], F32, tag="uT")
                    nc.scalar.activation(
                        out=ut[:], in_=ups[:], func=AF.Silu,
                        bias=ubu[:, ec:ec + 1], scale=1.0)
                    at2 = at2p.tile([128, T], F32, tag="aT2")
                    nc.sync.dma_start(out=at2, in_=aT_spill[e, ec])
                    gt = gtp.tile([128, T], F32R, tag="gT")
                    nc.vector.tensor_tensor(
                        out=gt[:], in0=ut[:], in1=at2[:], op=OP.mult)
                    g_tiles.append(gt)

                # ---- output projection + residual ----
                for tci in range(TC):
                    yt = yp.tile([128, H], F32, tag="y")
                    xr = xrp.tile([128, H], F32, tag="xr")
                    nc.sync.dma_start(
                        out=xr, in_=xres_d[e, tci * 128:(tci + 1) * 128, :])
                    for hs in range(H // 512):
                        yps = ps2.tile([128, 512], F32, tag="ps")
                        for ec in range(EC):
                            nc.tensor.matmul(
                                yps[:],
                                g_tiles[ec][:, tci * 128:(tci + 1) * 128],
                                wo[:, ec, hs * 512:(hs + 1) * 512],
                                start=(ec == 0), stop=(ec == EC - 1))
                        nc.vector.tensor_tensor(
                            out=yt[:, hs * 512:(hs + 1) * 512], in0=yps[:],
                            in1=xr[:, hs * 512:(hs + 1) * 512], op=OP.add)
                    nc.sync.dma_start(
                        out=y_d[e, tci * 128:(tci + 1) * 128, :], in_=yt[:])

    return nc


_BUILD_CACHE = {}


def _get_nc(with_vbias):
    key = bool(with_vbias)
    if key not in _BUILD_CACHE:
        nc = bacc.Bacc("TRN2", target_bir_lowering=False)
        _emit(nc, with_vbias)
        nc.compile()
        _BUILD_CACHE[key] = nc
    return _BUILD_CACHE[key]


def _rope_tables():
    """Rope sin/cos tables, computed with jax-on-cpu float32 ops exactly as
    the reference does (sin/cos of large fp32 arguments are implementation-
    sensitive, so matching op-for-op matters)."""
    import jax
    import jax.numpy as jnp

    cpu = jax.devices("cpu")[0]
    with jax.default_device(cpu):
        half = S // 2
        pos = jnp.arange(T, dtype=jnp.float32)
        inv_freq = 10000.0 ** (jnp.arange(half, dtype=jnp.float32) / half)
        sinusoid = pos[:, None] * inv_freq[None, :]          # [T, half]
        sin = np.asarray(jnp.sin(sinusoid)).astype(np.float32)
        cos = np.asarray(jnp.cos(sinusoid)).astype(np.float32)
    C = np.empty((S, T), np.float32)
    Sg = np.empty((S, T), np.float32)
    C[:half] = cos.T
    C[half:] = cos.T
    Sg[:half] = -sin.T   # q[s<64] = pre[s]*cos - pre[s+64]*sin
    Sg[half:] = sin.T    # q[s>=64] = pre[s]*cos + pre[s-64]*sin
    return C, Sg


def kernel(x, ln_w, ln_b, uv_w, uv_b, gamma, beta, w, o_w, o_b):
    x = np.ascontiguousarray(np.asarray(x, dtype=np.float32))
    ln_w = np.asarray(ln_w, np.float32)
    ln_b = np.asarray(ln_b, np.float32)
    uv_w = np.asarray(uv_w, np.float32)
    uv_b = np.asarray(uv_b, np.float32)
    gamma = np.asarray(gamma, np.float32)
    beta = np.asarray(beta, np.float32)
    w = np.asarray(w, np.float32)
    o_w = np.asarray(o_w, np.float32)
    o_b = np.asarray(o_b, np.float32)

    # ---- host-side folding ----
    # xn = core_ln(x) * ln_w + ln_b  =>  fold ln_w into uv_w cols, ln_b into bias
    w_eff = uv_w * ln_w[None, :]                 # [2E+S, H]
    uvb_eff = uv_b + uv_w @ ln_b                 # [2E+S]
    uv_wT = np.ascontiguousarray(w_eff.T)        # [H, 2E+S]
    wqkv = np.ascontiguousarray(uv_wT[:, E:])    # [H, E+S]  (v cols, then base)
    wu = np.ascontiguousarray(uv_wT[:, :E])      # [H, E]
    wo = np.ascontiguousarray(o_w.T)             # [E, H]

    # Toeplitz bias, transposed: biasT[j, i] = bias[i, j] = w[j - i + L - 1]
    idx = np.arange(T)
    biasT = np.ascontiguousarray(w[idx[:, None] - idx[None, :] + (L - 1)])

    ropeC, ropeS = _rope_tables()

    inv_sqrt_s = np.float32(1.0 / np.sqrt(np.float32(S)))
    gb = np.stack([gamma[0] * inv_sqrt_s, beta[0] * inv_sqrt_s,
                   gamma[1], beta[1]], axis=1).astype(np.float32)  # [S, 4]

    ubu = np.ascontiguousarray(
        uvb_eff[:E].reshape(EC, 128).T).astype(np.float32)  # [128, EC]
    ubb = uvb_eff[2 * E:].reshape(S, 1).astype(np.float32)
    vb = uvb_eff[E:2 * E].reshape(1, E).astype(np.float32)
    with_vbias = bool(np.any(vb))

    xres = x if not np.any(o_b) else (x + o_b[None, None, :]).astype(np.float32)

    nc = _get_nc(with_vbias)

    shared = {
        "wqkv_in": wqkv, "wu_in": wu, "wo_in": wo, "biasT_in": biasT,
        "ropeC_in": ropeC, "ropeS_in": ropeS, "gb_in": gb,
        "ubu_in": ubu, "ubb_in": ubb, "vb_in": vb,
    }
    in_maps = []
    for c in range(NCORES):
        sl = slice(c * BPC, (c + 1) * BPC)
        m = dict(shared)
        m["x_in"] = np.ascontiguousarray(x[sl])
        m["xres_in"] = np.ascontiguousarray(xres[sl])
        in_maps.append(m)

    res = run_bass_kernel_spmd(nc, in_maps, core_ids=list(range(NCORES)))
    out = np.concatenate([r["y_out"] for r in res.results], axis=0)
    return out
